# revision 23
# baseline (speedup 1.0000x reference)
"""Trainium2 Bass kernel for BiomechanicalConstraintModule.

Strategy (pure data parallel, batch sharded 8 ways):
  - Per core: 8 batches x 243 frames x 17 joints, C=128 features.
  - Super-tile = 27 frames = 459 joint-rows. 72 super-tiles per core.
  - Features are loaded row-major, transposed on the PE (via identity
    matmul) into C-on-partitions layout, cast to bf16.
  - Layer-1 matmuls are weight-stationary bf16 streaming F^T; the bone
    endpoint gather + concat-matmul is folded into gathered access
    patterns on the matmul moving operand with PSUM accumulation.
  - Tiny layer-2 matmuls use the hidden-activations-as-stationary trick
    to produce row-major outputs directly; biases are added with K=1
    ones-matmuls accumulated in PSUM.
  - Pose path (bone lengths / joint angles) runs frames-on-partitions
    with strided vector ops; arccos(x) = pi/2 - arctan(x*rsqrt(1-x^2)).
  - Time-mean for the action head accumulates per-joint column sums on
    the vector engine; the whole action MLP runs on-device per core.
"""

import numpy as np
import ml_dtypes

import concourse.bass as bass
import concourse.mybir as mybir
from concourse.ap import AP
from concourse.tile import TileContext
from concourse.bass_utils import run_bass_kernel_spmd

# ---------------------------------------------------------------- constants
B, T, J, C = 64, 243, 17, 128
NCORES = 8
B_SH = B // NCORES                  # batches per core
FR = B_SH * T                       # frames per core (1944)
ROWS = FR * J                       # joint rows per core (33048)
FR_ST = 27                          # frames per super-tile
COLS = FR_ST * J                    # 459
BONE_COLS = FR_ST * 15              # 405
ST_PER_B = T // FR_ST               # 9
N_ST = B_SH * ST_PER_B              # 72
SUBS = [(0, 128), (128, 128), (256, 128), (384, COLS - 384)]  # row chunks
BSUBS = [(0, 120), (120, 120), (240, 30), (270, 120), (390, 15)]

F32 = mybir.dt.float32
BF16 = mybir.dt.bfloat16
ACTF = mybir.ActivationFunctionType
BF = ml_dtypes.bfloat16


def _patch_tile_tail_drain():
    """The Tile tail drain can carry more semaphore waits than walrus
    accepts on one instruction; spread them over single-wait NOPs."""
    from concourse import tile as tile_mod
    from concourse.vector_clock import ScopedClock

    MAX_WAITS = 1
    MAX_WAITS_DMA = 1

    def _spill_excess_waits(self):
        """Walrus accepts only a few sem waits per instruction (and just
        one on HWDGE direct-2D DMAs); move the excess onto same-engine
        NOPs inserted just before the target."""
        nc = self.nc
        cur = nc.cur_bb.bb
        cur_name = cur.name
        spill_map = {}
        for bb in nc.m.functions[0].blocks:
            if bb.name == cur_name:
                continue
            for ins in list(bb.instructions):
                si = ins.sync_info
                if si is None:
                    continue
                lim = (
                    MAX_WAITS_DMA
                    if "DMA" in type(ins).__name__
                    else MAX_WAITS
                )
                w = list(si.on_wait)
                if len(w) <= lim:
                    continue
                keep, rest = w[:lim], w[lim:]
                base = len(cur.instructions)
                nops = []
                for i in range(0, len(rest), MAX_WAITS):
                    n = nc.engines[ins.engine].nop(
                        nofuse=True, hint="wait_spill"
                    )
                    n.ins.sync_info = type(si)(
                        on_wait=rest[i: i + MAX_WAITS], on_update=[]
                    )
                    nops.append(n.ins)
                # remove the freshly appended nops from the current bb
                tail = list(cur.instructions)
                assert len(tail) - base == len(nops)
                cur.instructions = tail[:base]
                si.on_wait = keep
                spill_map.setdefault(bb.name, []).append((ins.name, nops))
        for bb in nc.m.functions[0].blocks:
            if bb.name not in spill_map:
                continue
            targets = dict(spill_map[bb.name])
            new = []
            for ins in bb.instructions:
                if ins.name in targets:
                    new.extend(targets[ins.name])
                new.append(ins)
            bb.instructions = new

    def _drain_and_barrier(self, tick_clock, wait_clock):
        _spill_excess_waits(self)
        probe = self.nc.sync.nop(nofuse=True, hint="tail_wait_probe")
        wait_clock.add_sem_waits(
            probe.ins, ScopedClock({None: tick_clock.global_clock})
        )
        si = probe.ins.sync_info
        waits = list(si.on_wait) if si is not None else []
        if len(waits) > 1:
            probe.ins.sync_info.on_wait = waits[:1]
            for w in waits[1:]:
                n = self.nc.sync.nop(nofuse=True, hint="tail_wait")
                n.ins.sync_info = type(si)(on_wait=[w], on_update=[])
        self.nc.sync.drain()
        self.nc.all_engine_barrier()
        assert self.sems is not None
        popped = self.nc._tile_sem_poison_stack.pop()
        assert popped is self._sem_poison
        self.nc.clear_and_free_semaphores(list(self.sems.allocated().values()))
        self.nc.all_engine_barrier()

    tile_mod.TileContext._drain_and_barrier = _drain_and_barrier


def build_program(n_b=B_SH):
    _patch_tile_tail_drain()
    fr = n_b * T
    rows = fr * J
    n_st = n_b * ST_PER_B

    nc = bass.Bass("TRN2", target_bir_lowering=False, debug=False,
                   num_devices=NCORES)

    # ------------------------------------------------------------- dram io
    feat = nc.dram_tensor("feat", [rows, C], F32, kind="ExternalInput")
    pose = nc.dram_tensor("pose", [fr, J * 3], F32, kind="ExternalInput")
    ident = nc.dram_tensor("ident", [128, 128], F32, kind="ExternalInput")
    aw1b = nc.dram_tensor("aw1b", [128, 64], BF16, kind="ExternalInput")
    bw1t = nc.dram_tensor("bw1t", [128, 128], BF16, kind="ExternalInput")
    bw1u = nc.dram_tensor("bw1u", [128, 128], BF16, kind="ExternalInput")
    aw2r = nc.dram_tensor("aw2r", [128, 3], BF16, kind="ExternalInput")
    bw2b = nc.dram_tensor("bw2b", [128, 1], BF16, kind="ExternalInput")
    ones1 = nc.dram_tensor("ones1", [1, 128], BF16, kind="ExternalInput")
    ab2x8 = nc.dram_tensor("ab2x8", [1, 24], BF16, kind="ExternalInput")
    bb2x4 = nc.dram_tensor("bb2x4", [1, 5], BF16, kind="ExternalInput")
    ab1r = nc.dram_tensor("ab1r", [128, 1], F32, kind="ExternalInput")
    bb1c = nc.dram_tensor("bb1c", [128, 1], F32, kind="ExternalInput")
    cw1b = nc.dram_tensor("cw1b", [J * C, 64], BF16, kind="ExternalInput")
    cb1c = nc.dram_tensor("cb1c", [64, 1], F32, kind="ExternalInput")
    cw2b = nc.dram_tensor("cw2b", [64, 8], BF16, kind="ExternalInput")
    cb2r = nc.dram_tensor("cb2r", [1, 8], BF16, kind="ExternalInput")

    paO = nc.dram_tensor("paO", [rows, 3], F32, kind="ExternalOutput")
    pbO = nc.dram_tensor("pbO", [n_st * BONE_COLS], F32, kind="ExternalOutput")
    aaO = nc.dram_tensor("aaO", [fr, J], F32, kind="ExternalOutput")
    abO = nc.dram_tensor("abO", [fr, 15], F32, kind="ExternalOutput")
    lgO = nc.dram_tensor("lgO", [n_b, 8], F32, kind="ExternalOutput")

    with TileContext(nc) as tc:
        with (
            tc.tile_pool(name="wts", bufs=1) as wp,
            tc.tile_pool(name="frm", bufs=3) as pF,
            tc.tile_pool(name="ftb", bufs=3) as pFT,
            tc.tile_pool(name="h1", bufs=2) as pH1,
            tc.tile_pool(name="bh", bufs=2) as pBH,
            tc.tile_pool(name="outs", bufs=3) as pOut,
            tc.tile_pool(name="msum", bufs=1) as pMs,
            tc.tile_pool(name="mpart", bufs=2) as pMp,
            tc.tile_pool(name="pose", bufs=1) as pPo,
            tc.tile_pool(name="ppt", bufs=2, space="PSUM") as ppT,
            tc.tile_pool(name="ppa1", bufs=2, space="PSUM") as ppA1,
            tc.tile_pool(name="ppbh", bufs=2, space="PSUM") as ppBH,
            tc.tile_pool(name="ppsm", bufs=1, space="PSUM") as ppSm,
        ):
            # ---------------------------------------------------- weights
            def wtile(dram, shape, dtype):
                t = wp.tile(shape, dtype, tag=dram.name)
                nc.sync.dma_start(out=t[:, :], in_=dram[:, :])
                return t

            idT = wtile(ident, [128, 128], F32)
            aw1S = wtile(aw1b, [128, 64], BF16)
            bw1tS = wtile(bw1t, [128, 128], BF16)
            bw1uS = wtile(bw1u, [128, 128], BF16)
            aw2S = wtile(aw2r, [128, 3], BF16)
            bw2S = wtile(bw2b, [128, 1], BF16)
            onesS = wtile(ones1, [1, 128], BF16)
            ab2S = wtile(ab2x8, [1, 24], BF16)
            bb2S = wtile(bb2x4, [1, 5], BF16)
            ab1S = wtile(ab1r, [128, 1], F32)
            bb1S = wtile(bb1c, [128, 1], F32)
            cb1S = wtile(cb1c, [64, 1], F32)
            cw2S = wtile(cw2b, [64, 8], BF16)
            cb2S = wtile(cb2r, [1, 8], BF16)
            cw1S = wp.tile([128, J * 64], BF16)
            nc.sync.dma_start(
                out=cw1S[:, :],
                in_=cw1b[:, :].rearrange("(j p) m -> p j m", p=128),
            )

            # ---------------------------------------------------- pose path
            gfull = fr // 128
            gtail = fr - gfull * 128
            ngrp = gfull + (1 if gtail else 0)
            poS = pPo.tile([128, ngrp * 51], F32)
            vbS = pPo.tile([128, ngrp * 45], F32)
            sqS = pPo.tile([128, ngrp * 45], F32)
            blS = pPo.tile([128, ngrp * 15], F32)
            anS = pPo.tile([128, ngrp * 17], F32)
            tmpS = pPo.tile([128, ngrp * 12], F32)

            if gtail:
                nc.gpsimd.memset(poS[:, gfull * 51:], 0.0)
            if gfull:
                nc.sync.dma_start(
                    out=poS[:, :].rearrange("p (g x) -> p g x", x=51)[:, :gfull, :],
                    in_=pose[: gfull * 128, :].rearrange("(g p) x -> p g x", p=128),
                )
            if gtail:
                nc.sync.dma_start(
                    out=poS[:gtail, gfull * 51: gfull * 51 + 51],
                    in_=pose[gfull * 128:, :],
                )

            po3 = poS[:, :].rearrange("p (g x) -> p g x", x=51)  # [128,g,51]
            vb3 = vbS[:, :].rearrange("p (g x) -> p g x", x=45)  # [128,g,45]
            sq3 = sqS[:, :].rearrange("p (g x) -> p g x", x=45)
            bl3 = blS[:, :].rearrange("p (g x) -> p g x", x=15)
            an3 = anS[:, :].rearrange("p (g x) -> p g x", x=17)
            tp3 = tmpS[:, :].rearrange("p (g x) -> p g x", x=12)

            # bone vectors: child joints 1..15 minus parents.
            # bones k: child = k+1; parent = 0 for k%3==0 else child-1.
            # piece 1: k%3 in {1,2}: vb[k] = po[3(k+1)..] - po[3k..]
            vb_gk = vbS[:, :].rearrange(
                "p (g q k x) -> p g q k x", q=5, k=3, x=3
            )  # bone index = 3q+k
            po_j = po3  # joint j coords at 3j+c
            # children of piece1 bones (k=1,2 in group q): joints 3q+2, 3q+3
            ch1 = po_j[:, :, 6:51].rearrange("p g (q k x) -> p g q k x", k=3, x=3)[
                :, :, :, 0:2, :
            ]  # joints 3q+2 (x3 offset 6): wait layout check below
            # parents of piece1 bones: joints 3q+1, 3q+2
            pa1 = po_j[:, :, 3:48].rearrange("p g (q k x) -> p g q k x", k=3, x=3)[
                :, :, :, 0:2, :
            ]
            nc.vector.tensor_tensor(
                out=vb_gk[:, :, :, 1:3, :],
                in0=ch1,
                in1=pa1,
                op=mybir.AluOpType.subtract,
            )
            # piece 2: k%3==0 bones: child = joint 3q+1, parent = joint 0
            ch2 = po_j[:, :, 3:48].rearrange("p g (q x) -> p g q x", x=9)[
                :, :, :, 0:3
            ]  # joints 3q+1
            pa2 = po_j[:, :, 0:3].unsqueeze(2).broadcast_to([128, ngrp, 5, 3])
            nc.vector.tensor_tensor(
                out=vb_gk[:, :, :, 0:1, :].squeeze(3),
                in0=ch2,
                in1=pa2,
                op=mybir.AluOpType.subtract,
            )
            # squared components and bone lengths
            nc.vector.tensor_tensor(
                out=sq3, in0=vb3, in1=vb3, op=mybir.AluOpType.mult
            )
            sq_kx = sqS[:, :].rearrange("p (g k x) -> p g k x", k=15, x=3)
            nc.vector.tensor_tensor(
                out=bl3,
                in0=sq_kx[:, :, :, 0],
                in1=sq_kx[:, :, :, 1],
                op=mybir.AluOpType.add,
            )
            nc.vector.tensor_tensor(
                out=bl3,
                in0=bl3,
                in1=sq_kx[:, :, :, 2],
                op=mybir.AluOpType.add,
            )
            nc.scalar.activation(blS[:, :], blS[:, :], ACTF.Sqrt)
            # bl now = |bone vec|; DMA actual bone lengths out
            ab_src = blS[:, :].rearrange("p (g x) -> p g x", x=15)
            if gfull:
                nc.sync.dma_start(
                    out=AP(abO, 0, [[15, 128], [15 * 128, gfull], [1, 15]]),
                    in_=ab_src[:, :gfull, :],
                )
            if gtail:
                nc.sync.dma_start(
                    out=AP(abO, 15 * 128 * gfull, [[15, gtail], [1, 15]]),
                    in_=ab_src[:gtail, gfull, :],
                )

            # joint angles at joints {5,8,11,14} (sorted order):
            # v1 = bone (3a+4): joints 3a+4 <- 3a+5... v1 bone idx k1 = 3a+4
            # v2 = bone idx k2 = 3a+5  (a = 0..3)
            # NOTE bone k connects parent->child with child = k+1:
            #  angle joint j = 5+3a: v1 = po[j]-po[j-1] = bone j-1 = 4+3a
            #                  v2 = po[j+1]-po[j] = bone j = 5+3a
            # bones 3q+1 (q=1..4) = {4,7,10,13}; bones 3q+2 = {5,8,11,14}
            v1 = vb_gk[:, :, 1:5, 1, :]
            v2 = vb_gk[:, :, 1:5, 2, :]
            bl_qk = blS[:, :].rearrange("p (g q k) -> p g q k", q=5, k=3)
            n1 = bl_qk[:, :, 1:5, 1]
            n2 = bl_qk[:, :, 1:5, 2]
            # tmp layout per group: 12 = [dot(4) | d1(4) | d2(4)]
            dotv = tp3[:, :, 0:4]
            d1 = tp3[:, :, 4:8]
            d2 = tp3[:, :, 8:12]
            # dot = sum v1*v2
            prod = pPo.tile([128, ngrp * 12], F32)
            pr3 = prod[:, :].rearrange("p (g x) -> p g x", x=12)
            pr_ax = prod[:, :].rearrange("p (g a x) -> p g a x", a=4, x=3)
            nc.vector.tensor_tensor(
                out=pr_ax, in0=v1, in1=v2, op=mybir.AluOpType.mult
            )
            nc.vector.tensor_tensor(
                out=dotv, in0=pr_ax[:, :, :, 0], in1=pr_ax[:, :, :, 1],
                op=mybir.AluOpType.add,
            )
            nc.vector.tensor_tensor(
                out=dotv, in0=dotv, in1=pr_ax[:, :, :, 2],
                op=mybir.AluOpType.add,
            )
            # denom = (n1+eps)*(n2+eps); r = clip(dot/denom)
            nc.vector.tensor_scalar_add(out=d1, in0=n1, scalar1=1e-10)
            nc.vector.tensor_scalar_add(out=d2, in0=n2, scalar1=1e-10)
            nc.vector.tensor_tensor(
                out=d1, in0=d1, in1=d2, op=mybir.AluOpType.mult
            )
            nc.vector.reciprocal(out=d1, in_=d1)
            nc.vector.tensor_tensor(
                out=dotv, in0=dotv, in1=d1, op=mybir.AluOpType.mult
            )
            nc.vector.tensor_scalar_min(out=dotv, in0=dotv, scalar1=1.0 - 1e-7)
            nc.vector.tensor_scalar_max(out=dotv, in0=dotv, scalar1=-1.0 + 1e-7)
            # arccos via range-reduced arctan (ACT arctan domain is
            # [-pi/2, pi/2]): u = sqrt(1-x^2), theta = arctan(mn/mx) with
            # mn = min(|x|, u), mx = max(|x|, u) so the argument is <= 1;
            # phi = |x|<=u ? pi/2-theta : theta; arccos = x<0 ? pi-phi : phi.
            TT, TS = nc.vector.tensor_tensor, nc.vector.tensor_scalar
            OP = mybir.AluOpType
            tA = pr3[:, :, 0:4]
            tB = pr3[:, :, 4:8]
            tC = pr3[:, :, 8:12]
            TT(out=d1, in0=dotv, in1=dotv, op=OP.mult)       # x^2
            TS(out=d2, in0=d1, scalar1=-1.0, scalar2=1.0,
               op0=OP.mult, op1=OP.add)                      # u^2 = 1-x^2
            TT(out=tA, in0=d1, in1=d2, op=OP.min)            # mn^2
            TT(out=tB, in0=d1, in1=d2, op=OP.max)            # mx^2
            nc.vector.reciprocal(out=tB, in_=tB)
            TT(out=tA, in0=tA, in1=tB, op=OP.mult)           # (mn/mx)^2
            nc.scalar.activation(tA, tA, ACTF.Sqrt)          # mn/mx
            nc.scalar.activation(tA, tA, ACTF.Arctan)        # theta
            TT(out=tC, in0=d1, in1=d2, op=OP.is_le)          # x^2 <= u^2
            TS(out=tB, in0=tA, scalar1=-2.0, scalar2=float(np.pi / 2),
               op0=OP.mult, op1=OP.add)                      # pi/2 - 2*theta
            TT(out=tB, in0=tC, in1=tB, op=OP.mult)
            TT(out=tA, in0=tA, in1=tB, op=OP.add)            # phi
            TS(out=tC, in0=dotv, scalar1=0.0, scalar2=None, op0=OP.is_lt)
            TS(out=tB, in0=tA, scalar1=-2.0, scalar2=float(np.pi),
               op0=OP.mult, op1=OP.add)                      # pi - 2*phi
            TT(out=tB, in0=tC, in1=tB, op=OP.mult)
            # actual_angles tile: zeros except joints 5,8,11,14
            nc.gpsimd.memset(anS[:, :], 0.0)
            an_j = anS[:, :].rearrange("p (g j) -> p g j", j=17)
            an_tgt = an_j[:, :, 5:17].rearrange("p g (a x) -> p g a x", x=3)[
                :, :, :, 0
            ]
            TT(out=an_tgt, in0=tA, in1=tB, op=OP.add)
            if gfull:
                nc.sync.dma_start(
                    out=AP(aaO, 0, [[17, 128], [17 * 128, gfull], [1, 17]]),
                    in_=an_j[:, :gfull, :],
                )
            if gtail:
                nc.sync.dma_start(
                    out=AP(aaO, 17 * 128 * gfull, [[17, gtail], [1, 17]]),
                    in_=an_j[:gtail, gfull, :],
                )

            # ---------------------------------------------------- main loop
            msumS = pMs.tile([128, n_b * J], F32)
            nc.gpsimd.memset(msumS[:, :], 0.0)

            pair_psum = None
            pair_h1 = None
            for t in range(n_st):
                bidx = t // ST_PER_B
                s_in_pair = t % 2
                r0 = t * COLS

                # load row-major features (4 stacked subtiles)
                frm = pF.tile([128, 512], F32)
                for s, (o, w) in enumerate(SUBS):
                    nc.sync.dma_start(
                        out=frm[:w, 128 * s: 128 * s + 128],
                        in_=feat[r0 + o: r0 + o + w, :],
                    )
                # PE transpose -> psum [C=128, 459] f32
                ftp = ppT.tile([128, COLS], F32, padded_shape=[128, 512])
                for s, (o, w) in enumerate(SUBS):
                    nc.tensor.transpose(
                        ftp[:, o: o + w],
                        frm[:w, 128 * s: 128 * s + 128],
                        idT[:w, :w],
                    )
                # cast copy -> sbuf bf16 (alternate engine per tile)
                ftb = pFT.tile([128, COLS], BF16)
                if t % 2 == 0:
                    nc.scalar.activation(ftb[:, :], ftp[:, :], ACTF.Copy)
                else:
                    nc.vector.tensor_copy(ftb[:, :], ftp[:, :])

                ft3 = ftb[:, :].rearrange("p (f j) -> p f j", j=J)

                # time-sum for action head: reduce over frames
                mp = pMp.tile([128, J], F32)
                nc.vector.tensor_reduce(
                    out=mp[:, :],
                    in_=ft3.transpose([0, 2, 1]),
                    axis=mybir.AxisListType.X,
                    op=mybir.AluOpType.add,
                )
                msl = msumS[:, bidx * J: (bidx + 1) * J]
                nc.vector.tensor_tensor(
                    out=msl, in0=msl, in1=mp[:, :], op=mybir.AluOpType.add
                )

                # angle layer 1 (pairs share one psum on partition halves)
                if s_in_pair == 0:
                    pair_psum = ppA1.tile([128, COLS], F32, tag="a1", padded_shape=[128, 512])
                nc.tensor.matmul(
                    out=pair_psum[64 * s_in_pair: 64 * s_in_pair + 64, :],
                    lhsT=aw1S[:, :],
                    rhs=ftb[:, :],
                    start=True,
                    stop=True,
                )

                # bone layer 1: gathered rhs, accumulate top+bottom halves
                # bone hidden, compacted psum layout (all matmul outs are
                # contiguous): cols [0:270) = bones 3q+1+k (col 10f+2q+k),
                # cols [270:405) = bones 3q (col 270+5f+q).
                bhp = ppBH.tile([128, BONE_COLS], F32, padded_shape=[128, 512])
                p2 = bhp[:, 0:270].rearrange("p (f q k) -> p f q k", q=5, k=2)
                p1 = bhp[:, 270:405].rearrange("p (f q) -> p f q", q=5)
                chld2 = ft3[:, :, 2:17].rearrange(
                    "p f (q k) -> p f q k", k=3
                )[:, :, :, 0:2]  # joints 3q+2, 3q+3
                par2 = ft3[:, :, 1:16].rearrange(
                    "p f (q k) -> p f q k", k=3
                )[:, :, :, 0:2]  # joints 3q+1, 3q+2
                chld1 = ft3[:, :, 1:16].rearrange(
                    "p f (q k) -> p f q k", k=3
                )[:, :, :, 0]  # joints 3q+1
                par1 = ft3[:, :, 0:1].broadcast_to([128, FR_ST, 5])  # joint 0
                nc.tensor.matmul(
                    out=p2, lhsT=bw1uS[:, :], rhs=chld2,
                    start=True, stop=False, skip_group_check=True,
                )
                nc.tensor.matmul(
                    out=p2, lhsT=bw1tS[:, :], rhs=par2,
                    start=False, stop=False, skip_group_check=True,
                )
                nc.tensor.matmul(
                    out=p1, lhsT=bw1uS[:, :], rhs=chld1,
                    start=False, stop=False, skip_group_check=True,
                )
                nc.tensor.matmul(
                    out=p1, lhsT=bw1tS[:, :], rhs=par1,
                    start=False, stop=True, skip_group_check=True,
                )
                # bone gelu -> sbuf bf16
                bhb = pBH.tile([128, BONE_COLS], BF16)
                nc.scalar.activation(
                    bhb[:, :], bhp[:, :], ACTF.Gelu, bias=bb1S[:, 0:1]
                )

                # bone layer 2: chunks stationary, bias via ones-matmul.
                # Chunk boundaries align to frame subgroups so the output
                # DMA can un-shuffle the compacted layout affinely.
                pbp = ppSm.tile([128, 5], F32, tag="pb", padded_shape=[128, 512])
                nc.tensor.matmul(
                    out=pbp[:, :],
                    lhsT=onesS[0:1, :],
                    rhs=bb2S[0:1, :],
                    start=True,
                    stop=False,
                    skip_group_check=True,
                )
                for c, (o, w) in enumerate(BSUBS):
                    nc.tensor.matmul(
                        out=pbp[:w, c: c + 1],
                        lhsT=bhb[:, o: o + w],
                        rhs=bw2S[:, :],
                        start=False,
                        stop=(c == len(BSUBS) - 1),
                        skip_group_check=True,
                    )
                pbs = pOut.tile([128, 5], F32, tag="pbs")
                nc.scalar.activation(pbs[:, :], pbp[:, :], ACTF.Relu)
                for c, (o, w) in enumerate(BSUBS):
                    if o < 270:  # P2 chunk: rows are (f, q, k) -> bone 3q+1+k
                        f0, nf = o // 10, w // 10
                        dst = AP(
                            pbO,
                            t * BONE_COLS + 15 * f0 + 1,
                            [[15, nf], [3, 5], [1, 2]],
                        )
                    else:  # P1 chunk: rows are (f, q) -> bone 3q
                        f0, nf = (o - 270) // 5, w // 5
                        dst = AP(
                            pbO,
                            t * BONE_COLS + 15 * f0,
                            [[15, nf], [3, 5]],
                        )
                    nc.sync.dma_start(out=dst, in_=pbs[:w, c: c + 1])

                # angle gelu + layer 2, once per pair (or final odd tile)
                last_of_pair = s_in_pair == 1 or t == n_st - 1
                if not last_of_pair:
                    pair_h1 = None
                    continue
                np_parts = 64 * (s_in_pair + 1)
                h1b = pH1.tile([128, COLS], BF16, tag="h1b")
                nc.scalar.activation(
                    h1b[:np_parts, :],
                    pair_psum[:np_parts, :],
                    ACTF.Gelu,
                    bias=ab1S[:np_parts, 0:1],
                )
                pap = ppSm.tile([128, 24], F32, tag="pa", padded_shape=[128, 512])
                nc.tensor.matmul(
                    out=pap[:, :],
                    lhsT=onesS[0:1, :],
                    rhs=ab2S[0:1, :],
                    start=True,
                    stop=False,
                    skip_group_check=True,
                )
                nmm = (s_in_pair + 1) * 4
                k = 0
                for s in range(s_in_pair + 1):
                    for c, (o, w) in enumerate(SUBS):
                        k += 1
                        nc.tensor.matmul(
                            out=pap[:w, 3 * (4 * s + c): 3 * (4 * s + c) + 3],
                            lhsT=h1b[64 * s: 64 * s + 64, o: o + w],
                            rhs=aw2S[64 * s: 64 * s + 64, :],
                            start=False,
                            stop=(k == nmm),
                            skip_group_check=True,
                        )
                pas = pOut.tile([128, 24], F32, tag="pas")
                nc.scalar.activation(pas[:, :], pap[:, :], ACTF.Copy)
                pr0 = (t - s_in_pair) * COLS
                pa4 = pas[:, :].rearrange("p (s c x) -> p s c x", s=2, c=4)
                for s in range(s_in_pair + 1):
                    nc.sync.dma_start(
                        out=AP(
                            paO,
                            (pr0 + COLS * s) * 3,
                            [[3, 128], [128 * 3, 3], [1, 3]],
                        ),
                        in_=pa4[:, s, 0:3, :],
                    )
                    nc.sync.dma_start(
                        out=AP(
                            paO,
                            (pr0 + COLS * s + 384) * 3,
                            [[3, COLS - 384], [1, 3]],
                        ),
                        in_=pa4[: COLS - 384, s, 3, :],
                    )
                pair_psum = None
                pair_h1 = None

            # ---------------------------------------------------- action MLP
            msb = pMs.tile([128, n_b * J], BF16)
            nc.scalar.activation(msb[:, :], msumS[:, :], ACTF.Copy)
            ms3 = msb[:, :].rearrange("p (b j) -> p b j", j=J)
            c1p = ppSm.tile([64, n_b], F32, tag="pa", padded_shape=[128, 512])
            for j in range(J):
                nc.tensor.matmul(
                    out=c1p[:, :],
                    lhsT=cw1S[:, 64 * j: 64 * j + 64],
                    rhs=ms3[:, :, j],
                    start=(j == 0),
                    stop=(j == J - 1),
                    skip_group_check=True,
                )
            hcS = pMs.tile([64, n_b], BF16)
            nc.scalar.activation(
                hcS[:, :], c1p[:, :], ACTF.Gelu, bias=cb1S[:, 0:1]
            )
            c2p = ppSm.tile([n_b, 8], F32, tag="pb", padded_shape=[128, 512])
            nc.tensor.matmul(
                out=c2p[:, :],
                lhsT=onesS[0:1, :n_b],
                rhs=cb2S[0:1, :],
                start=True,
                stop=False,
                skip_group_check=True,
            )
            nc.tensor.matmul(
                out=c2p[:, :],
                lhsT=hcS[:, :],
                rhs=cw2S[:, :],
                start=False,
                stop=True,
                skip_group_check=True,
            )
            lgS = pMs.tile([n_b, 8], F32)
            nc.scalar.activation(lgS[:, :], c2p[:, :], ACTF.Copy)
            nc.sync.dma_start(out=lgO[:, :], in_=lgS[:, :])

    return nc


_PROGRAM_CACHE = {}


def _get_program(n_b=B_SH):
    if n_b not in _PROGRAM_CACHE:
        _PROGRAM_CACHE[n_b] = build_program(n_b)
    return _PROGRAM_CACHE[n_b]


def make_in_map(features, pose3d, weights, core):
    """Per-core input dict. features [B,T,J,C] f32, pose3d [B,T,J,3]."""
    f = np.ascontiguousarray(
        features[core * B_SH: (core + 1) * B_SH], dtype=np.float32
    ).reshape(ROWS, C)
    p = np.ascontiguousarray(
        pose3d[core * B_SH: (core + 1) * B_SH], dtype=np.float32
    ).reshape(FR, J * 3)
    return {"feat": f, "pose": p, **weights}


def make_weights(aw1, ab1, aw2, ab2, bw1, bb1, bw2, bb2, cw1, cb1, cw2, cb2):
    bf = lambda x: np.ascontiguousarray(x, dtype=np.float32).astype(BF)
    f32 = lambda x: np.ascontiguousarray(x, dtype=np.float32)
    return {
        "ident": f32(np.eye(128)),
        "aw1b": bf(aw1),
        "bw1t": bf(bw1[:C]),
        "bw1u": bf(bw1[C:]),
        "aw2r": bf(np.vstack([aw2, aw2])),
        "bw2b": bf(bw2),
        "ones1": bf(np.ones((1, 128))),
        "ab2x8": bf(np.tile(ab2, 8)[None, :]),
        "bb2x4": bf(np.full((1, 5), float(bb2[0]))),
        "ab1r": f32(np.concatenate([ab1, ab1])[:, None]),
        "bb1c": f32(bb1[:, None]),
        "cw1b": bf(cw1 / float(T)),
        "cb1c": f32(cb1[:, None]),
        "cw2b": bf(cw2),
        "cb2r": bf(cb2[None, :]),
    }


def assemble(results):
    """results: list of 8 per-core output dicts -> reference output tuple."""
    pa = np.concatenate(
        [r["paO"].reshape(B_SH, T, J, 3) for r in results], axis=0
    )
    aa = np.concatenate(
        [r["aaO"].reshape(B_SH, T, J, 1) for r in results], axis=0
    )
    pb = np.concatenate(
        [r["pbO"].reshape(B_SH, T, 15, 1) for r in results], axis=0
    )
    ab = np.concatenate(
        [r["abO"].reshape(B_SH, T, 15, 1) for r in results], axis=0
    )
    lg = np.concatenate([r["lgO"] for r in results], axis=0)
    return pa, aa, pb, ab, lg


def kernel(features, pose3d, aw1, ab1, aw2, ab2, bw1, bb1, bw2, bb2,
           cw1, cb1, cw2, cb2):
    nc = _get_program()
    weights = make_weights(
        aw1, ab1, aw2, ab2, bw1, bb1, bw2, bb2, cw1, cb1, cw2, cb2
    )
    in_maps = [
        make_in_map(features, pose3d, weights, core) for core in range(NCORES)
    ]
    res = run_bass_kernel_spmd(nc, in_maps, list(range(NCORES)))
    return assemble(res.results)


# revision 31
# speedup vs baseline: 1.1831x; 1.1831x over previous
"""Trainium2 Bass kernel for BiomechanicalConstraintModule.

Strategy (pure data parallel, batch sharded 8 ways):
  - Per core: 8 batches x 243 frames x 17 joints, C=128 features.
  - Super-tile = 27 frames = 459 joint-rows. 72 super-tiles per core.
  - Features loaded row-major with ONE 256KB DMA per super-tile (tail
    overlap-padded), transposed on the PE (identity matmul) into
    C-on-partitions layout, cast to bf16 on ACT/DVE alternately.
  - Layer-1 matmuls are weight-stationary bf16 streaming F^T; the bone
    endpoint gather + concat is folded into gathered access patterns on
    the moving operand with PSUM accumulation (compacted bone layout so
    every matmul out is contiguous).
  - Layer-2 matmuls are weight-stationary producing transposed outputs;
    four super-tiles are packed into one PSUM bank at partition offsets
    {0,32,64,96}, biased+activated with ONE ACT op per group, staged in
    SBUF, and written with a handful of large strided DMAs to scratch
    DRAM. The host un-transposes / un-shuffles during unshard.
  - Pose path (bone lengths / joint angles) runs frames-on-partitions
    with strided vector ops; arccos via range-reduced arctan.
  - Time-mean for the action head: per-super-tile strided reduce on DVE;
    the whole action MLP runs on-device per core.
"""

import numpy as np
import ml_dtypes

import concourse.bass as bass
import concourse.mybir as mybir
from concourse.ap import AP
from concourse.tile import TileContext
from concourse.bass_utils import run_bass_kernel_spmd

# ---------------------------------------------------------------- constants
B, T, J, C = 64, 243, 17, 128
NCORES = 8
B_SH = B // NCORES                  # batches per core
FR = B_SH * T                       # frames per core (1944)
ROWS = FR * J                       # joint rows per core (33048)
FR_ST = 27                          # frames per super-tile
COLS = FR_ST * J                    # 459
BONE_COLS = FR_ST * 15              # 405
ST_PER_B = T // FR_ST               # 9
N_ST = B_SH * ST_PER_B              # 72
GC = 6                              # psum-groups (of 4 tiles) per out chunk

F32 = mybir.dt.float32
BF16 = mybir.dt.bfloat16
ACTF = mybir.ActivationFunctionType
BF = ml_dtypes.bfloat16

# packed bf16 weight-block column layout
WB_AW1 = (0, 64)
WB_BW1T = (64, 192)
WB_BW1U = (192, 320)
WB_AW2 = (320, 323)
WB_BW2 = (323, 324)
WB_CW2 = (324, 332)
WB_ONES = (332, 460)
WB_CB2 = (460, 468)
WB_ONESW = (468, 927)     # ones row, width 459
WB_AB2R = (927, 1055)     # ab2 pattern row (partition-bias transposed)
WB_BB2R = (1055, 1183)    # bb2 pattern row
WB_N = 1183
# packed f32 weight-block columns
WF_AB1, WF_BB1, WF_CB1, WF_AB2R4, WF_BB2R4, WF_ID0 = 0, 1, 2, 3, 4, 5
WF_N = 5 + 128


def _patch_tile_tail_drain():
    """Walrus accepts very few semaphore waits per instruction (1 on
    several opcode structs). Spill excess waits onto adjacent same-engine
    NOPs, and chunk the tail drain's waits the same way."""
    from concourse import tile as tile_mod
    from concourse.vector_clock import ScopedClock

    MAX_WAITS = 1

    def _spill_excess_waits(self):
        nc = self.nc
        cur = nc.cur_bb.bb
        cur_name = cur.name
        spill_map = {}
        for bb in nc.m.functions[0].blocks:
            if bb.name == cur_name:
                continue
            for ins in list(bb.instructions):
                si = ins.sync_info
                if si is None:
                    continue
                w = list(si.on_wait)
                if len(w) <= MAX_WAITS:
                    continue
                keep, rest = w[:MAX_WAITS], w[MAX_WAITS:]
                base = len(cur.instructions)
                nops = []
                for i in range(0, len(rest), MAX_WAITS):
                    n = nc.engines[ins.engine].nop(
                        nofuse=True, hint="wait_spill"
                    )
                    n.ins.sync_info = type(si)(
                        on_wait=rest[i: i + MAX_WAITS], on_update=[]
                    )
                    nops.append(n.ins)
                tail = list(cur.instructions)
                assert len(tail) - base == len(nops)
                cur.instructions = tail[:base]
                si.on_wait = keep
                spill_map.setdefault(bb.name, []).append((ins.name, nops))
        for bb in nc.m.functions[0].blocks:
            if bb.name not in spill_map:
                continue
            targets = dict(spill_map[bb.name])
            new = []
            for ins in bb.instructions:
                if ins.name in targets:
                    new.extend(targets[ins.name])
                new.append(ins)
            bb.instructions = new

    def _drain_and_barrier(self, tick_clock, wait_clock):
        _spill_excess_waits(self)
        probe = self.nc.sync.nop(nofuse=True, hint="tail_wait_probe")
        wait_clock.add_sem_waits(
            probe.ins, ScopedClock({None: tick_clock.global_clock})
        )
        si = probe.ins.sync_info
        waits = list(si.on_wait) if si is not None else []
        if len(waits) > 1:
            probe.ins.sync_info.on_wait = waits[:1]
            for w in waits[1:]:
                n = self.nc.sync.nop(nofuse=True, hint="tail_wait")
                n.ins.sync_info = type(si)(on_wait=[w], on_update=[])
        self.nc.sync.drain()
        self.nc.all_engine_barrier()
        assert self.sems is not None
        popped = self.nc._tile_sem_poison_stack.pop()
        assert popped is self._sem_poison
        self.nc.clear_and_free_semaphores(list(self.sems.allocated().values()))
        self.nc.all_engine_barrier()

    tile_mod.TileContext._drain_and_barrier = _drain_and_barrier


def build_program(n_b=B_SH):
    _patch_tile_tail_drain()
    fr = n_b * T
    rows = fr * J
    n_st = n_b * ST_PER_B
    rows_pad = n_st * COLS + 53        # last super-tile loads a full 512
    n_grp4 = -(-n_st // 4)             # psum groups of up to 4 tiles
    padt = 459 * 4 * n_grp4            # padded transposed-out columns
    padb = 405 * 4 * n_grp4

    nc = bass.Bass("TRN2", target_bir_lowering=False, debug=False,
                   num_devices=NCORES)

    # ------------------------------------------------------------- dram io
    feat = nc.dram_tensor("feat", [rows_pad, C], F32, kind="ExternalInput")
    pose = nc.dram_tensor("pose", [fr, J * 3], F32, kind="ExternalInput")
    wbf = nc.dram_tensor("wbf", [128, WB_N], BF16, kind="ExternalInput")
    wf32 = nc.dram_tensor("wf32", [128, WF_N], F32, kind="ExternalInput")
    cw1b = nc.dram_tensor("cw1b", [J * C, 64], BF16, kind="ExternalInput")

    paT = nc.dram_tensor("paT", [3, padt], F32, kind="ExternalOutput")
    pbC = nc.dram_tensor("pbC", [padb], F32, kind="ExternalOutput")
    aaO = nc.dram_tensor("aaO", [fr, J], F32, kind="ExternalOutput")
    abO = nc.dram_tensor("abO", [fr, 15], F32, kind="ExternalOutput")
    lgO = nc.dram_tensor("lgO", [n_b, 8], F32, kind="ExternalOutput")

    with TileContext(nc) as tc:
        with (
            tc.tile_pool(name="wts", bufs=1) as wp,
            tc.tile_pool(name="frm", bufs=3) as pF,
            tc.tile_pool(name="ftb", bufs=3) as pFT,
            tc.tile_pool(name="h1", bufs=2) as pH1,
            tc.tile_pool(name="bh", bufs=2) as pBH,
            tc.tile_pool(name="patsb", bufs=2) as pPA,
            tc.tile_pool(name="pbtsb", bufs=2) as pPB,
            tc.tile_pool(name="msum", bufs=1) as pMs,
            tc.tile_pool(name="mpart", bufs=2) as pMp,
            tc.tile_pool(name="pose", bufs=1) as pPo,
            tc.tile_pool(name="ppt", bufs=2, space="PSUM") as ppT,
            tc.tile_pool(name="ppa1", bufs=2, space="PSUM") as ppA1,
            tc.tile_pool(name="ppbh", bufs=1, space="PSUM") as ppBH,
            tc.tile_pool(name="ppsm", bufs=1, space="PSUM") as ppSm,
        ):
            # ---------------------------------------------------- weights
            wbfS = wp.tile([128, WB_N], BF16)
            nc.sync.dma_start(out=wbfS[:, :], in_=wbf[:, :])
            wf32S = wp.tile([128, WF_N], F32)
            nc.sync.dma_start(out=wf32S[:, :], in_=wf32[:, :])
            cw1S = wp.tile([128, J * 64], BF16)
            nc.sync.dma_start(
                out=cw1S[:, :],
                in_=cw1b[:, :].rearrange("(j p) m -> p j m", p=128),
            )
            aw1S = wbfS[:, WB_AW1[0]: WB_AW1[1]]
            bw1tS = wbfS[:, WB_BW1T[0]: WB_BW1T[1]]
            bw1uS = wbfS[:, WB_BW1U[0]: WB_BW1U[1]]
            aw2S = wbfS[:, WB_AW2[0]: WB_AW2[1]]
            bw2S = wbfS[:, WB_BW2[0]: WB_BW2[1]]
            cw2S = wbfS[0:64, WB_CW2[0]: WB_CW2[1]]
            onesS = wbfS[0:1, WB_ONES[0]: WB_ONES[1]]
            cb2S = wbfS[0:1, WB_CB2[0]: WB_CB2[1]]
            onesWS = wbfS[0:1, WB_ONESW[0]: WB_ONESW[1]]
            ab2rS = wbfS[0:1, WB_AB2R[0]: WB_AB2R[1]]
            bb2rS = wbfS[0:1, WB_BB2R[0]: WB_BB2R[1]]
            ab1S = wf32S[:, WF_AB1: WF_AB1 + 1]
            bb1S = wf32S[:, WF_BB1: WF_BB1 + 1]
            cb1S = wf32S[0:64, WF_CB1: WF_CB1 + 1]
            ab2r4S = wf32S[:, WF_AB2R4: WF_AB2R4 + 1]
            bb2r4S = wf32S[:, WF_BB2R4: WF_BB2R4 + 1]
            idT = wf32S[:, WF_ID0: WF_ID0 + 128]

            # ---------------------------------------------------- pose path
            gfull = fr // 128
            gtail = fr - gfull * 128
            ngrp = gfull + (1 if gtail else 0)
            poS = pPo.tile([128, ngrp * 51], F32)
            vbS = pPo.tile([128, ngrp * 45], F32)
            sqS = pPo.tile([128, ngrp * 45], F32)
            blS = pPo.tile([128, ngrp * 15], F32)
            anS = pPo.tile([128, ngrp * 17], F32)
            tmpS = pPo.tile([128, ngrp * 12], F32)
            prod = pPo.tile([128, ngrp * 12], F32)

            if gtail:
                nc.gpsimd.memset(poS[:, gfull * 51:], 0.0)
            if gfull:
                nc.sync.dma_start(
                    out=poS[:, :].rearrange("p (g x) -> p g x", x=51)[:, :gfull, :],
                    in_=pose[: gfull * 128, :].rearrange("(g p) x -> p g x", p=128),
                )
            if gtail:
                nc.sync.dma_start(
                    out=poS[:gtail, gfull * 51: gfull * 51 + 51],
                    in_=pose[gfull * 128:, :],
                )

            po3 = poS[:, :].rearrange("p (g x) -> p g x", x=51)
            vb3 = vbS[:, :].rearrange("p (g x) -> p g x", x=45)
            sq3 = sqS[:, :].rearrange("p (g x) -> p g x", x=45)
            bl3 = blS[:, :].rearrange("p (g x) -> p g x", x=15)
            tp3 = tmpS[:, :].rearrange("p (g x) -> p g x", x=12)
            pr3 = prod[:, :].rearrange("p (g x) -> p g x", x=12)

            # bone vectors: bones 3q+{1,2} then bones 3q (parent = joint 0)
            vb_gk = vbS[:, :].rearrange(
                "p (g q k x) -> p g q k x", q=5, k=3, x=3
            )
            ch1 = po3[:, :, 6:51].rearrange(
                "p g (q k x) -> p g q k x", k=3, x=3
            )[:, :, :, 0:2, :]
            pa1 = po3[:, :, 3:48].rearrange(
                "p g (q k x) -> p g q k x", k=3, x=3
            )[:, :, :, 0:2, :]
            nc.vector.tensor_tensor(
                out=vb_gk[:, :, :, 1:3, :], in0=ch1, in1=pa1,
                op=mybir.AluOpType.subtract,
            )
            ch2 = po3[:, :, 3:48].rearrange("p g (q x) -> p g q x", x=9)[
                :, :, :, 0:3
            ]
            pa2 = po3[:, :, 0:3].unsqueeze(2).broadcast_to([128, ngrp, 5, 3])
            nc.vector.tensor_tensor(
                out=vb_gk[:, :, :, 0:1, :].squeeze(3), in0=ch2, in1=pa2,
                op=mybir.AluOpType.subtract,
            )
            nc.vector.tensor_tensor(
                out=sq3, in0=vb3, in1=vb3, op=mybir.AluOpType.mult
            )
            sq_kx = sqS[:, :].rearrange("p (g k x) -> p g k x", k=15, x=3)
            nc.vector.tensor_tensor(
                out=bl3, in0=sq_kx[:, :, :, 0], in1=sq_kx[:, :, :, 1],
                op=mybir.AluOpType.add,
            )
            nc.vector.tensor_tensor(
                out=bl3, in0=bl3, in1=sq_kx[:, :, :, 2],
                op=mybir.AluOpType.add,
            )
            nc.scalar.activation(blS[:, :], blS[:, :], ACTF.Sqrt)
            ab_src = blS[:, :].rearrange("p (g x) -> p g x", x=15)
            if gfull:
                nc.sync.dma_start(
                    out=AP(abO, 0, [[15, 128], [15 * 128, gfull], [1, 15]]),
                    in_=ab_src[:, :gfull, :],
                )
            if gtail:
                nc.sync.dma_start(
                    out=AP(abO, 15 * 128 * gfull, [[15, gtail], [1, 15]]),
                    in_=ab_src[:gtail, gfull, :],
                )

            # joint angles at joints {5,8,11,14}: v1 = bone 3q+1, v2 = bone
            # 3q+2 for q=1..4
            v1 = vb_gk[:, :, 1:5, 1, :]
            v2 = vb_gk[:, :, 1:5, 2, :]
            dotv = tp3[:, :, 0:4]
            d1 = tp3[:, :, 4:8]
            d2 = tp3[:, :, 8:12]
            pr_ax = prod[:, :].rearrange("p (g a x) -> p g a x", a=4, x=3)
            nc.vector.tensor_tensor(
                out=pr_ax, in0=v1, in1=v2, op=mybir.AluOpType.mult
            )
            nc.vector.tensor_tensor(
                out=dotv, in0=pr_ax[:, :, :, 0], in1=pr_ax[:, :, :, 1],
                op=mybir.AluOpType.add,
            )
            nc.vector.tensor_tensor(
                out=dotv, in0=dotv, in1=pr_ax[:, :, :, 2],
                op=mybir.AluOpType.add,
            )
            bl_qk = blS[:, :].rearrange("p (g q k) -> p g q k", q=5, k=3)
            n1 = bl_qk[:, :, 1:5, 1]
            n2 = bl_qk[:, :, 1:5, 2]
            nc.vector.tensor_scalar_add(out=d1, in0=n1, scalar1=1e-10)
            nc.vector.tensor_scalar_add(out=d2, in0=n2, scalar1=1e-10)
            nc.vector.tensor_tensor(
                out=d1, in0=d1, in1=d2, op=mybir.AluOpType.mult
            )
            nc.vector.reciprocal(out=d1, in_=d1)
            nc.vector.tensor_tensor(
                out=dotv, in0=dotv, in1=d1, op=mybir.AluOpType.mult
            )
            nc.vector.tensor_scalar_min(out=dotv, in0=dotv, scalar1=1.0 - 1e-7)
            nc.vector.tensor_scalar_max(out=dotv, in0=dotv, scalar1=-1.0 + 1e-7)
            # arccos via range-reduced arctan (ACT arctan domain [-pi/2,pi/2])
            TT, TS = nc.vector.tensor_tensor, nc.vector.tensor_scalar
            OP = mybir.AluOpType
            tA = pr3[:, :, 0:4]
            tB = pr3[:, :, 4:8]
            tC = pr3[:, :, 8:12]
            TT(out=d1, in0=dotv, in1=dotv, op=OP.mult)       # x^2
            TS(out=d2, in0=d1, scalar1=-1.0, scalar2=1.0,
               op0=OP.mult, op1=OP.add)                      # u^2 = 1-x^2
            TT(out=tA, in0=d1, in1=d2, op=OP.min)            # mn^2
            TT(out=tB, in0=d1, in1=d2, op=OP.max)            # mx^2
            nc.vector.reciprocal(out=tB, in_=tB)
            TT(out=tA, in0=tA, in1=tB, op=OP.mult)           # (mn/mx)^2
            nc.scalar.activation(tA, tA, ACTF.Sqrt)
            nc.scalar.activation(tA, tA, ACTF.Arctan)        # theta
            TT(out=tC, in0=d1, in1=d2, op=OP.is_le)          # x^2 <= u^2
            TS(out=tB, in0=tA, scalar1=-2.0, scalar2=float(np.pi / 2),
               op0=OP.mult, op1=OP.add)
            TT(out=tB, in0=tC, in1=tB, op=OP.mult)
            TT(out=tA, in0=tA, in1=tB, op=OP.add)            # phi
            TS(out=tC, in0=dotv, scalar1=0.0, scalar2=None, op0=OP.is_lt)
            TS(out=tB, in0=tA, scalar1=-2.0, scalar2=float(np.pi),
               op0=OP.mult, op1=OP.add)
            TT(out=tB, in0=tC, in1=tB, op=OP.mult)
            nc.gpsimd.memset(anS[:, :], 0.0)
            an_j = anS[:, :].rearrange("p (g j) -> p g j", j=17)
            an_tgt = an_j[:, :, 5:17].rearrange("p g (a x) -> p g a x", x=3)[
                :, :, :, 0
            ]
            TT(out=an_tgt, in0=tA, in1=tB, op=OP.add)
            if gfull:
                nc.sync.dma_start(
                    out=AP(aaO, 0, [[17, 128], [17 * 128, gfull], [1, 17]]),
                    in_=an_j[:, :gfull, :],
                )
            if gtail:
                nc.sync.dma_start(
                    out=AP(aaO, 17 * 128 * gfull, [[17, gtail], [1, 17]]),
                    in_=an_j[:gtail, gfull, :],
                )

            # ---------------------------------------------------- main loop
            msumS = pMs.tile([128, n_b * J], F32)
            nc.gpsimd.memset(msumS[:, :], 0.0)

            pair_psum = None
            pap4 = None
            pbp4 = None
            patS = None
            pbtS = None
            n_chunks = -(-n_grp4 // GC)
            for t in range(n_st):
                bidx = t // ST_PER_B
                s_in_pair = t % 2
                ki = t % 4
                grp = t // 4
                g_loc = grp % GC
                r0 = t * COLS

                # one 256KB DMA: [p, s, c] -> rows r0+128s+p
                frm = pF.tile([128, 512], F32)
                nc.sync.dma_start(
                    out=frm[:, :],
                    in_=AP(feat, r0 * 128,
                           [[128, 128], [16384, 4], [1, 128]]),
                )
                ftp = ppT.tile([128, 512], F32)
                for s in range(4):
                    nc.tensor.transpose(
                        ftp[:, 128 * s: 128 * s + 128],
                        frm[:, 128 * s: 128 * s + 128],
                        idT[:, :],
                    )
                ftb = pFT.tile([128, COLS], BF16)
                if t % 2 == 0:
                    nc.scalar.activation(ftb[:, :], ftp[:, :COLS], ACTF.Copy)
                else:
                    nc.vector.tensor_copy(ftb[:, :], ftp[:, :COLS])

                ft3 = ftb[:, :].rearrange("p (f j) -> p f j", j=J)

                # time-sum for action head
                mp = pMp.tile([128, J], F32)
                nc.vector.tensor_reduce(
                    out=mp[:, :],
                    in_=ft3.transpose([0, 2, 1]),
                    axis=mybir.AxisListType.X,
                    op=mybir.AluOpType.add,
                )
                msl = msumS[:, bidx * J: (bidx + 1) * J]
                nc.vector.tensor_tensor(
                    out=msl, in0=msl, in1=mp[:, :], op=mybir.AluOpType.add
                )

                # angle layer 1 (pairs share one psum on partition halves)
                if s_in_pair == 0:
                    pair_psum = ppA1.tile(
                        [128, COLS], F32, tag="a1", padded_shape=[128, 512]
                    )
                nc.tensor.matmul(
                    out=pair_psum[64 * s_in_pair: 64 * s_in_pair + 64, :],
                    lhsT=aw1S,
                    rhs=ftb[:, :],
                    start=True,
                    stop=True,
                )

                # bone layer 1, compacted psum layout: cols [0:270) bones
                # 3q+1+k (col 10f+2q+k), cols [270:405) bones 3q.
                bhp = ppBH.tile(
                    [128, BONE_COLS], F32, padded_shape=[128, 512]
                )
                p2 = bhp[:, 0:270].rearrange("p (f q k) -> p f q k", q=5, k=2)
                p1 = bhp[:, 270:405].rearrange("p (f q) -> p f q", q=5)
                chld2 = ft3[:, :, 2:17].rearrange(
                    "p f (q k) -> p f q k", k=3
                )[:, :, :, 0:2]
                par2 = ft3[:, :, 1:16].rearrange(
                    "p f (q k) -> p f q k", k=3
                )[:, :, :, 0:2]
                chld1 = ft3[:, :, 1:16].rearrange(
                    "p f (q k) -> p f q k", k=3
                )[:, :, :, 0]
                par1 = ft3[:, :, 0:1].broadcast_to([128, FR_ST, 5])
                nc.tensor.matmul(
                    out=p2, lhsT=bw1uS, rhs=chld2,
                    start=True, stop=False, skip_group_check=True,
                )
                nc.tensor.matmul(
                    out=p2, lhsT=bw1tS, rhs=par2,
                    start=False, stop=False, skip_group_check=True,
                )
                nc.tensor.matmul(
                    out=p1, lhsT=bw1uS, rhs=chld1,
                    start=False, stop=False, skip_group_check=True,
                )
                nc.tensor.matmul(
                    out=p1, lhsT=bw1tS, rhs=par1,
                    start=False, stop=True, skip_group_check=True,
                )
                bhb = pBH.tile([128, BONE_COLS], BF16)
                nc.scalar.activation(
                    bhb[:, :], bhp[:, :], ACTF.Gelu, bias=bb1S
                )

                # bone layer 2 (weight-stationary, transposed out), four
                # super-tiles packed per psum bank at partitions {0,32,64,96}
                if ki == 0:
                    pbp4 = ppSm.tile(
                        [128, BONE_COLS], F32, tag="pb",
                        padded_shape=[128, 512],
                    )
                    nc.tensor.matmul(
                        out=pbp4[:, :],
                        lhsT=bb2rS,
                        rhs=onesWS[0:1, :BONE_COLS],
                        start=True,
                        stop=False,
                        skip_group_check=True,
                    )
                nc.tensor.matmul(
                    out=pbp4[32 * ki: 32 * ki + 1, :],
                    lhsT=bw2S,
                    rhs=bhb[:, :],
                    start=False,
                    stop=(ki == 3 or t == n_st - 1),
                    skip_group_check=True,
                    tile_position=(0, 32 * ki),
                )
                if ki == 3 or t == n_st - 1:
                    if g_loc == 0:
                        pbtS = pPB.tile([128, 405 * GC], F32, tag="pbt")
                    nc.scalar.activation(
                        pbtS[:, 405 * g_loc: 405 * g_loc + 405],
                        pbp4[:, :],
                        ACTF.Relu,
                    )
                    if g_loc == GC - 1 or t == n_st - 1:
                        t0 = (grp // GC) * GC * 4
                        gcnt = min(GC, n_grp4 - (grp // GC) * GC)
                        for k in range(4):
                            src = pbtS[
                                32 * k: 32 * k + 1, : 405 * gcnt
                            ].rearrange("p (g x) -> p g x", x=405)
                            nc.sync.dma_start(
                                out=AP(
                                    pbC, 405 * (t0 + k),
                                    [[1, 1], [405 * 4, gcnt], [1, 405]],
                                ),
                                in_=src,
                            )

                # angle gelu + layer 2, once per pair (or final odd tile)
                if not (s_in_pair == 1 or t == n_st - 1):
                    continue
                np_parts = 64 * (s_in_pair + 1)
                h1b = pH1.tile([128, COLS], BF16, tag="h1b")
                nc.scalar.activation(
                    h1b[:np_parts, :],
                    pair_psum[:np_parts, :],
                    ACTF.Gelu,
                    bias=ab1S[:np_parts, :],
                )
                for s in range(s_in_pair + 1):
                    kk = ki - s_in_pair + s
                    if kk == 0:
                        pap4 = ppSm.tile(
                            [128, COLS], F32, tag="pa",
                            padded_shape=[128, 512],
                        )
                        nc.tensor.matmul(
                            out=pap4[:, :],
                            lhsT=ab2rS,
                            rhs=onesWS[0:1, :COLS],
                            start=True,
                            stop=False,
                            skip_group_check=True,
                        )
                    nc.tensor.matmul(
                        out=pap4[32 * kk: 32 * kk + 3, :],
                        lhsT=aw2S[64 * s: 64 * s + 64, :],
                        rhs=h1b[64 * s: 64 * s + 64, :],
                        start=False,
                        stop=False,
                        skip_group_check=True,
                        tile_position=(64 * s, 32 * kk),
                    )
                if ki == 3 or t == n_st - 1:
                    if g_loc == 0:
                        patS = pPA.tile([128, 459 * GC], F32, tag="pat")
                    nc.scalar.activation(
                        patS[:, 459 * g_loc: 459 * g_loc + 459],
                        pap4[:, :],
                        ACTF.Identity,
                    )
                    if g_loc == GC - 1 or t == n_st - 1:
                        t0 = (grp // GC) * GC * 4
                        gcnt = min(GC, n_grp4 - (grp // GC) * GC)
                        for k in range(4):
                            for c in range(3):
                                src = patS[
                                    32 * k + c: 32 * k + c + 1, : 459 * gcnt
                                ].rearrange("p (g x) -> p g x", x=459)
                                nc.sync.dma_start(
                                    out=AP(
                                        paT,
                                        c * padt + 459 * (t0 + k),
                                        [[1, 1], [459 * 4, gcnt], [1, 459]],
                                    ),
                                    in_=src,
                                )

            # ---------------------------------------------------- action MLP
            msb = pMs.tile([128, n_b * J], BF16)
            nc.scalar.activation(msb[:, :], msumS[:, :], ACTF.Copy)
            ms3 = msb[:, :].rearrange("p (b j) -> p b j", j=J)
            c1p = ppSm.tile(
                [64, n_b], F32, tag="pa", padded_shape=[128, 512]
            )
            for j in range(J):
                nc.tensor.matmul(
                    out=c1p[:, :],
                    lhsT=cw1S[:, 64 * j: 64 * j + 64],
                    rhs=ms3[:, :, j],
                    start=(j == 0),
                    stop=(j == J - 1),
                    skip_group_check=True,
                )
            hcS = pMs.tile([64, n_b], BF16)
            nc.scalar.activation(hcS[:, :], c1p[:, :], ACTF.Gelu, bias=cb1S)
            c2p = ppSm.tile(
                [n_b, 8], F32, tag="pb", padded_shape=[128, 512]
            )
            nc.tensor.matmul(
                out=c2p[:, :],
                lhsT=onesS[0:1, :n_b],
                rhs=cb2S,
                start=True,
                stop=False,
                skip_group_check=True,
            )
            nc.tensor.matmul(
                out=c2p[:, :],
                lhsT=hcS[:, :],
                rhs=cw2S,
                start=False,
                stop=True,
                skip_group_check=True,
            )
            lgS = pMs.tile([n_b, 8], F32)
            nc.scalar.activation(lgS[:, :], c2p[:, :], ACTF.Copy)
            nc.sync.dma_start(out=lgO[:, :], in_=lgS[:, :])

    return nc


_PROGRAM_CACHE = {}


def _get_program(n_b=B_SH):
    if n_b not in _PROGRAM_CACHE:
        _PROGRAM_CACHE[n_b] = build_program(n_b)
    return _PROGRAM_CACHE[n_b]


def make_in_map(features, pose3d, weights, core, n_b=B_SH):
    fr = n_b * T
    rows = fr * J
    n_st = n_b * ST_PER_B
    rows_pad = n_st * COLS + 53
    f = np.ascontiguousarray(
        features[core * n_b: (core + 1) * n_b], dtype=np.float32
    ).reshape(rows, C)
    fpad = np.zeros((rows_pad, C), np.float32)
    fpad[:rows] = f
    p = np.ascontiguousarray(
        pose3d[core * n_b: (core + 1) * n_b], dtype=np.float32
    ).reshape(fr, J * 3)
    return {"feat": fpad, "pose": p, **weights}


def make_weights(aw1, ab1, aw2, ab2, bw1, bb1, bw2, bb2, cw1, cb1, cw2, cb2):
    f32 = lambda x: np.ascontiguousarray(x, dtype=np.float32)
    wb = np.zeros((128, WB_N), np.float32)
    wb[:, WB_AW1[0]: WB_AW1[1]] = aw1
    wb[:, WB_BW1T[0]: WB_BW1T[1]] = bw1[:C]
    wb[:, WB_BW1U[0]: WB_BW1U[1]] = bw1[C:]
    wb[:, WB_AW2[0]: WB_AW2[1]] = np.vstack([aw2, aw2])
    wb[:, WB_BW2[0]: WB_BW2[1]] = bw2
    wb[:64, WB_CW2[0]: WB_CW2[1]] = cw2
    wb[0, WB_ONES[0]: WB_ONES[1]] = 1.0
    wb[0, WB_CB2[0]: WB_CB2[1]] = cb2
    wb[0, WB_ONESW[0]: WB_ONESW[1]] = 1.0
    for k in range(4):
        wb[0, WB_AB2R[0] + 32 * k: WB_AB2R[0] + 32 * k + 3] = ab2
        wb[0, WB_BB2R[0] + 32 * k] = float(bb2[0])
    wf = np.zeros((128, WF_N), np.float32)
    wf[:, WF_AB1] = np.concatenate([ab1, ab1])
    wf[:, WF_BB1] = bb1
    wf[:64, WF_CB1] = cb1
    for k in range(4):
        wf[32 * k: 32 * k + 3, WF_AB2R4] = ab2
        wf[32 * k, WF_BB2R4] = float(bb2[0])
    wf[:, WF_ID0: WF_ID0 + 128] = np.eye(128)
    return {
        "wbf": wb.astype(BF),
        "wf32": f32(wf),
        "cw1b": np.ascontiguousarray(cw1 / float(T), np.float32).astype(BF),
    }


def assemble(results, n_b=B_SH):
    """results: list of per-core output dicts -> reference output tuple."""
    n_st = n_b * ST_PER_B
    rows = n_st * COLS

    def _pa(r):
        x = r["paT"][:, :rows]            # [3, rows]
        return np.ascontiguousarray(x.T).reshape(n_b, T, J, 3)

    def _pb(r):
        x = r["pbC"][: n_st * 405].reshape(n_st, 405)
        out = np.empty((n_st, FR_ST, 15), np.float32)
        out[:, :, 1::3] = x[:, :270].reshape(n_st, FR_ST, 5, 2)[..., 0]
        out[:, :, 2::3] = x[:, :270].reshape(n_st, FR_ST, 5, 2)[..., 1]
        out[:, :, 0::3] = x[:, 270:].reshape(n_st, FR_ST, 5)
        return out.reshape(n_b, T, 15, 1)

    pa = np.concatenate([_pa(r) for r in results], axis=0)
    aa = np.concatenate(
        [r["aaO"].reshape(n_b, T, J, 1) for r in results], axis=0
    )
    pb = np.concatenate([_pb(r) for r in results], axis=0)
    ab = np.concatenate(
        [r["abO"].reshape(n_b, T, 15, 1) for r in results], axis=0
    )
    lg = np.concatenate([r["lgO"] for r in results], axis=0)
    return pa, aa, pb, ab, lg


def kernel(features, pose3d, aw1, ab1, aw2, ab2, bw1, bb1, bw2, bb2,
           cw1, cb1, cw2, cb2):
    nc = _get_program()
    weights = make_weights(
        aw1, ab1, aw2, ab2, bw1, bb1, bw2, bb2, cw1, cb1, cw2, cb2
    )
    in_maps = [
        make_in_map(features, pose3d, weights, core) for core in range(NCORES)
    ]
    res = run_bass_kernel_spmd(nc, in_maps, list(range(NCORES)))
    return assemble(res.results)
